# revision 1
# baseline (speedup 1.0000x reference)
"""Trainium2 Bass kernel for nn_AnnCloseModel (hydrology ANN closure model).

Reference per timestep t (serial scan over nt=365, carry yt (ngrid,1)):
    z_t  = where(isnan(y_obs_t), yhat_{t-1}, y_obs_t)     # fillObs
    h    = relu([x_t, z_t] @ Wi.T + bi)
    yhat_t = (h @ Wh.T + bh) @ Wo.T + bo

Algebraic folds (host-side):
  * No nonlinearity between Wh and Wo:  yhat = relu(.) @ Wc.T + bc,
    Wc = Wo@Wh (256,), bc = Wo@bh+bo (scalar).
  * z_t = y_clean_t + mask_t * yhat_{t-1}  (y_clean = nan_to_num(y), mask = isnan(y))
  * u = Wi16.T x + bi + wiy*(y_clean + mask*(py_prev + bc)) where py = yhat - bc
    -> K=18 matmul on pure inputs [y_clean; x(16); mask] with stationary rows
       [wiy; Wi16.T; bc*wiy], plus ONE accumulating K=1 matmul wiy (x) (mask.py_prev)
       whose moving operand is lane-aligned with the col-tiled mm3 output.
  * bi folded into the relu op (per-partition bias), bc added on host to outputs.

Device layout (per core; grid shard padded to 3840 = 2 halves x 4 groups x 480):
  hidden-on-partitions; grid on the free axis. Grid chunk (h, cg) occupies
  partition group 32*cg (rows +0 y_clean, +1..16 x, +17 mask) at free h*480.
  mm1/fb row-tiled at tile_position (32*cg, 0) (K<=32 -> 4 concurrent groups);
  mm3 (Wc dot) col-tiled at (0, 32*cg) so chunk cg's output lands on lane 32*cg.
Sharding: pure data parallelism over ngrid across 8 cores; no collectives.
"""

import os
import numpy as np

NT, NGRID, NX = 365, 30000, 16
HIDDEN = 256
NCORES = 8
GSH = 3840                     # padded grid rows per core
CH = 480                       # matmul free-dim chunk
NGRP = 4                       # partition groups (tile positions 32*cg)
NHALF = 2

_RELU_ACT = int(os.environ.get("RELU_ACT", "4"))   # of 8 relu ops, how many on ScalarE


def _legalize_sync(nc, max_waits=1):
    """This walrus build rejects instructions carrying more than one sync wait
    ("Too many sync wait commands"); hoist excess waits onto preceding NoOps."""
    import concourse.mybir as mybir

    n_new = 0
    for fn in nc.m.functions:
        for bb in fn.blocks:
            out = []
            changed = False
            for inst in bb.instructions:
                si = inst.sync_info
                if si is not None and si.on_wait and len(si.on_wait) > max_waits:
                    waits = list(si.on_wait)
                    head, tail = waits[:-max_waits], waits[-max_waits:]
                    for i, w in enumerate(head):
                        out.append(
                            mybir.InstNoOp(
                                name=f"{inst.name}-syncfix{i}",
                                sync_info=mybir.SyncInfo(on_wait=[w], on_update=[]),
                                bass_nofuse=True,
                                engine=inst.engine,
                            )
                        )
                        n_new += 1
                    inst.sync_info = mybir.SyncInfo(
                        on_wait=tail, on_update=list(si.on_update or [])
                    )
                    changed = True
                out.append(inst)
            if changed:
                bb.instructions = out
    return n_new


def _build_nc(nt, bc):
    from contextlib import ExitStack

    import concourse.bass as bass
    import concourse.mybir as mybir
    import concourse.tile as tile

    F32 = mybir.dt.float32
    BF16 = mybir.dt.bfloat16
    AF = mybir.ActivationFunctionType
    ALU = mybir.AluOpType

    nc = bass.Bass(trn_type="TRN2")
    xym = nc.dram_tensor("xym", (nt, NGRP, 19, 2 * CH), BF16, kind="ExternalInput")
    msk = nc.dram_tensor("msk", (nt, NGRP, 1, 2 * CH), BF16, kind="ExternalInput")
    w1d = nc.dram_tensor("w1", (128, 256), BF16, kind="ExternalInput")
    wcd = nc.dram_tensor("wc", (128, 64), BF16, kind="ExternalInput")
    outd = nc.dram_tensor("out", (nt, NGRP, NHALF, CH), F32, kind="ExternalOutput")

    with tile.TileContext(nc) as tc, ExitStack() as ctx:
        const = ctx.enter_context(tc.tile_pool(name="const", bufs=1))
        rhs_pool = ctx.enter_context(tc.tile_pool(name="rhs", bufs=4))
        ps_pool = ctx.enter_context(tc.tile_pool(name="ps", bufs=2, space="PSUM"))
        py_pool = ctx.enter_context(tc.tile_pool(name="py", bufs=4, space="PSUM"))
        r_pool = ctx.enter_context(tc.tile_pool(name="r", bufs=6))
        ym_pool = ctx.enter_context(tc.tile_pool(name="ym", bufs=4))
        yh_pool = ctx.enter_context(tc.tile_pool(name="yh", bufs=4))

        w1 = const.tile([128, 256], BF16)
        nc.sync.dma_start(w1[:, :], w1d[:, :])
        wc = const.tile([128, 64], BF16)
        nc.sync.dma_start(wc[:, :], wcd[:, :])
        # persistent mask tile, double-buffered by t parity; memset once so the
        # unused partition rows are 0.0 (they feed a full-width multiply).
        mka = const.tile([128, 2 * 2 * CH], BF16)
        nc.vector.memset(mka[:, :], 0.0)

        rhs_tiles = {}

        def load_step(t):
            rhs = rhs_pool.tile([128, 2 * CH], BF16, tag="rhs")
            mk = mka[:, (t % 2) * 2 * CH : (t % 2 + 1) * 2 * CH]
            for cg in range(NGRP):
                nc.sync.dma_start(rhs[32 * cg : 32 * cg + 19, :], xym[t, cg])
            mk_grp = mk.rearrange("(g s) n -> g s n", s=32)[:, 0:1, :]
            nc.sync.dma_start(mk_grp, msk[t])
            rhs_tiles[t] = rhs

        load_step(0)
        py_prev = {}   # half -> psum tile holding yhat-bc rows at {0,32,64,96}
        for t in range(nt):
            rhs = rhs_tiles.pop(t)
            mk = mka[:, (t % 2) * 2 * CH : (t % 2 + 1) * 2 * CH]
            if t + 1 < nt:
                load_step(t + 1)

            for h in range(NHALF):
                # feedback operand: yhm = mask_t * py_prev  (valid on lanes 32*cg)
                yhm = None
                if t > 0:
                    yhm = ym_pool.tile([128, CH], BF16, tag="ym")
                    nc.vector.tensor_mul(
                        yhm[:, :], py_prev[h][:, :], mk[:, h * CH : (h + 1) * CH]
                    )

                rtiles = {}
                relu_idx = 0
                for m in range(2):
                    for p in range(2):  # cg pair {2p, 2p+1}
                        ps = ps_pool.tile([128, 1024], F32, tag="ps")
                        for q in range(2):
                            cg = 2 * p + q
                            nc.tensor.matmul(
                                ps[:, q * 512 : q * 512 + CH],
                                w1[32 * cg : 32 * cg + 19, m * 128 : (m + 1) * 128],
                                rhs[32 * cg : 32 * cg + 19, h * CH : (h + 1) * CH],
                                start=True,
                                stop=(t == 0),
                                tile_position=(32 * cg, 0),
                                skip_group_check=True,
                            )
                            if t > 0:
                                nc.tensor.matmul(
                                    ps[:, q * 512 : q * 512 + CH],
                                    w1[32 * cg : 32 * cg + 1, m * 128 : (m + 1) * 128],
                                    yhm[32 * cg : 32 * cg + 1, :],
                                    start=False,
                                    stop=True,
                                    tile_position=(32 * cg, 0),
                                    skip_group_check=True,
                                )
                        # relu + bias: r = max(ps + bi, 0), PSUM -> SBUF
                        # (banks are 512 wide; quarters live at 512-offsets, the
                        # 480-wide views skip the 32 junk columns per bank)
                        r = r_pool.tile([128, 1024], BF16, tag="r")
                        ps_v = ps[:, :].rearrange("p (b j) -> p b j", j=512)[:, :, 0:CH]
                        r_v = r[:, :].rearrange("p (b j) -> p b j", j=512)[:, :, 0:CH]
                        if relu_idx < _RELU_ACT:
                            nc.scalar.activation(r_v, ps_v, AF.Relu)
                        else:
                            nc.vector.tensor_scalar_max(r_v, ps_v, 0.0)
                        relu_idx += 1
                        rtiles[(m, p)] = r

                # mm3: py = Wc . r (col-tiled; chunk cg -> lane 32*cg)
                py = py_pool.tile([128, CH], F32, tag="py")
                for p in range(2):
                    for q in range(2):
                        cg = 2 * p + q
                        for m in range(2):
                            nc.tensor.matmul(
                                py[32 * cg : 32 * cg + 32, :],
                                wc[:, m * 32 : (m + 1) * 32],
                                rtiles[(m, p)][:, q * 512 : q * 512 + CH],
                                start=(m == 0),
                                stop=(m == 1),
                                tile_position=(0, 32 * cg),
                                skip_group_check=True,
                            )
                py_prev[h] = py

                # store yhat = py + bc (PSUM -> SBUF copy, then DMA)
                yh = yh_pool.tile([128, CH], F32, tag="yh")
                nc.scalar.activation(yh[:, :], py[:, :], AF.Copy, bias=float(bc))
                yhv = yh[:, :].rearrange("(g s) n -> g s n", s=32)[:, 0, :]
                nc.sync.dma_start(outd[t, :, h, :], yhv)

    return nc


def _prep_core_inputs(x_c, y_c):
    """x_c (nt, GSH, 16) f32, y_c (nt, GSH) f32 (NaN = missing) -> xym, msk (bf16)."""
    import ml_dtypes

    nt = x_c.shape[0]
    xym = np.empty((nt, NGRP, 19, 2 * CH), dtype=np.float32)
    # grid index G = (h*NGRP + cg)*CH + j  <->  xym[t, cg, :, h*CH + j]
    xv = x_c.reshape(nt, NHALF, NGRP, CH, NX)  # [t, h, cg, j, f]
    xym[:, :, 1:17, :] = xv.transpose(0, 2, 4, 1, 3).reshape(nt, NGRP, NX, 2 * CH)
    yv = y_c.reshape(nt, NHALF, NGRP, CH)
    y_clean = np.nan_to_num(yv, nan=0.0, posinf=None, neginf=None)
    mask = np.isnan(yv).astype(np.float32)
    xym[:, :, 0, :] = y_clean.transpose(0, 2, 1, 3).reshape(nt, NGRP, 2 * CH)
    mk = mask.transpose(0, 2, 1, 3).reshape(nt, NGRP, 1, 2 * CH)
    xym[:, :, 17, :] = mk[:, :, 0, :]
    xym[:, :, 18, :] = 1.0
    # t=0 carry is exactly zero: no bc*wiy*mask contribution
    xym[0, :, 17, :] = 0.0
    return (
        np.ascontiguousarray(xym).astype(ml_dtypes.bfloat16),
        np.ascontiguousarray(mk).astype(ml_dtypes.bfloat16),
    )


def _prep_weights(Wi, bi, Wh, bh, Wo, bo):
    Wc = (Wo.astype(np.float64) @ Wh.astype(np.float64)).reshape(HIDDEN)
    bc = float(
        (Wo.astype(np.float64) @ bh.astype(np.float64) + bo.astype(np.float64))[0]
    )
    wiy = Wi[:, NX].astype(np.float64)
    # stationary rows per group: {0: wiy, 1..16: Wi16.T, 17: bc*wiy, 18: bi}
    W1full = np.empty((19, HIDDEN), dtype=np.float32)
    W1full[0] = wiy
    W1full[1:17] = Wi[:, :NX].T
    W1full[17] = (bc * wiy).astype(np.float32)
    W1full[18] = bi
    import ml_dtypes

    w1 = np.zeros((128, 256), dtype=ml_dtypes.bfloat16)
    for cg in range(NGRP):
        w1[32 * cg : 32 * cg + 19] = W1full.astype(ml_dtypes.bfloat16)
    # Wc replicated to 32 stationary columns per block so mm3 writes all
    # 128 psum partitions (avoids uninitialized lanes; same cycle cost)
    wcm = np.zeros((128, 64), dtype=ml_dtypes.bfloat16)
    wcm[:, 0:32] = Wc[:128, None].astype(ml_dtypes.bfloat16)
    wcm[:, 32:64] = Wc[128:, None].astype(ml_dtypes.bfloat16)
    bib = None  # bi folded into the ones-row of the stationary
    return w1, wcm, bib, bc


def kernel(x, y, Wi, bi, Wh, bh, Wo, bo):
    from concourse.bass_utils import run_bass_kernel_spmd

    x = np.asarray(x, dtype=np.float32)
    y = np.asarray(y, dtype=np.float32)
    nt = x.shape[0]
    ngrid = x.shape[1]

    w1, wcm, bib, bc = _prep_weights(
        np.asarray(Wi, np.float32),
        np.asarray(bi, np.float32),
        np.asarray(Wh, np.float32),
        np.asarray(bh, np.float32),
        np.asarray(Wo, np.float32),
        np.asarray(bo, np.float32),
    )

    gpc = ngrid // NCORES
    in_maps = []
    for c in range(NCORES):
        x_c = np.zeros((nt, GSH, NX), dtype=np.float32)
        y_c = np.zeros((nt, GSH), dtype=np.float32)
        x_c[:, :gpc] = x[:, c * gpc : (c + 1) * gpc, :]
        y_c[:, :gpc] = y[:, c * gpc : (c + 1) * gpc, 0]
        xym, mk = _prep_core_inputs(x_c, y_c)
        in_maps.append({"xym": xym, "msk": mk, "w1": w1, "wc": wcm})

    nc = _build_nc(nt, bc)
    _legalize_sync(nc)
    results = run_bass_kernel_spmd(nc, in_maps, core_ids=list(range(NCORES)))
    global _LAST_EXEC_NS, _LAST_RESULTS
    _LAST_EXEC_NS = results.exec_time_ns
    _LAST_RESULTS = results

    out = np.empty((nt, ngrid, 1), dtype=np.float32)
    for c in range(NCORES):
        # (nt, NGRP, NHALF, CH) -> (nt, GSH):  G = (h*NGRP+cg)*CH + j
        o = results.results[c]["out"].transpose(0, 2, 1, 3).reshape(nt, GSH)
        out[:, c * gpc : (c + 1) * gpc, 0] = o[:, :gpc]
    return out



# revision 7
# speedup vs baseline: 1.8679x; 1.8679x over previous
"""Trainium2 Bass kernel for nn_AnnCloseModel (hydrology ANN closure model).

Reference per timestep t (serial scan over nt=365, carry yt (ngrid,1)):
    z_t  = where(isnan(y_obs_t), yhat_{t-1}, y_obs_t)     # fillObs
    h    = relu([x_t, z_t] @ Wi.T + bi)
    yhat_t = (h @ Wh.T + bh) @ Wo.T + bo

Algebraic folds (host-side):
  * No nonlinearity between Wh and Wo:  yhat = relu(.) @ Wc.T + bc,
    Wc = Wo@Wh (256,), bc = Wo@bh+bo (scalar).
  * z_t = (y_clean + bc*mask) + mask*py_prev  (py = yhat - bc), so the whole
    step is ONE K=19 matmul per (hidden-half, grid-chunk) on moving rows
    [y0'; x(16); ones; fb] with stationary rows [wiy; Wi16.T; bi; wiy].
    fb = mask_t * py_{t-1} is computed on GPSIMD from the SBUF copy of py and
    placed into the rhs tile's row 18+32cg by a tiny SBUF->SBUF DMA
    (compute engines cannot write partition-strided APs; DMA can).
  * bc is added on the host after the run (device outputs py in bf16).

Device layout (per core; grid shard padded to 3840 = 2 halves x 4 groups x 480):
  hidden-on-partitions; grid on the free axis. Grid chunk (h, cg) occupies
  partition group 32*cg (rows +0 y0', +1..16 x, +17 ones, +18 fb) at free
  h*480.  mm1 row-tiled at tile_position (32*cg, 0); mm3 (Wc dot) col-tiled
  at (0, 32*cg) so chunk cg's output lands on lane band 32*cg (replicated
  over the 32 lanes, which lets strided views pick row 18+32cg directly).
  relu evacuation (PSUM->SBUF, the true bottleneck) is split between the
  Scalar (ACT) and Vector (DVE) engines.
Sharding: pure data parallelism over ngrid across 8 cores; no collectives.
"""

import os
import numpy as np

NT, NGRID, NX = 365, 30000, 16
HIDDEN = 256
NCORES = 8
GSH = 3840                     # padded grid rows per core
CH = 480                       # matmul free-dim chunk
NGRP = 4                       # partition groups (tile positions 32*cg)
NHALF = 2
PF = 4                         # DMA prefetch depth (steps)

# which (m,p) ps tiles get their relu on ACT (rest on DVE); 8 tiles/step
_ACT_TILES = int(os.environ.get("RELU_ACT_TILES", "4"))  # of 8


def _legalize_sync(nc, max_waits=1):
    """This walrus build rejects instructions carrying more than one sync wait
    ("Too many sync wait commands"); hoist excess waits onto preceding NoOps."""
    import concourse.mybir as mybir

    n_new = 0
    for fn in nc.m.functions:
        for bb in fn.blocks:
            out = []
            changed = False
            for inst in bb.instructions:
                si = inst.sync_info
                if si is not None and si.on_wait and len(si.on_wait) > max_waits:
                    waits = list(si.on_wait)
                    head, tail = waits[:-max_waits], waits[-max_waits:]
                    for i, w in enumerate(head):
                        out.append(
                            mybir.InstNoOp(
                                name=f"{inst.name}-syncfix{i}",
                                sync_info=mybir.SyncInfo(on_wait=[w], on_update=[]),
                                bass_nofuse=True,
                                engine=inst.engine,
                            )
                        )
                        n_new += 1
                    inst.sync_info = mybir.SyncInfo(
                        on_wait=tail, on_update=list(si.on_update or [])
                    )
                    changed = True
                out.append(inst)
            if changed:
                bb.instructions = out
    return n_new


def _build_nc(nt):
    from contextlib import ExitStack

    import concourse.bass as bass
    import concourse.mybir as mybir
    import concourse.tile as tile

    F32 = mybir.dt.float32
    BF16 = mybir.dt.bfloat16
    AF = mybir.ActivationFunctionType
    ALU = mybir.AluOpType

    nc = bass.Bass(trn_type="TRN2")
    # rows per group: 0=y0', 1..16=x, 17=ones  (fb row 18 is device-written)
    xym = nc.dram_tensor("xym", (nt, NGRP, 18, 2 * CH), BF16, kind="ExternalInput")
    # mask(t+1), laid out to match yh's 512-strided halves
    msk = nc.dram_tensor("msk", (nt, NGRP, NHALF, CH), BF16, kind="ExternalInput")
    w1d = nc.dram_tensor("w1", (128, 256), BF16, kind="ExternalInput")
    wcd = nc.dram_tensor("wc", (128, 64), BF16, kind="ExternalInput")
    outd = nc.dram_tensor("out", (nt, NGRP, NHALF, CH), BF16, kind="ExternalOutput")

    with tile.TileContext(nc) as tc, ExitStack() as ctx:
        const = ctx.enter_context(tc.tile_pool(name="const", bufs=1))
        rhs_pool = ctx.enter_context(tc.tile_pool(name="rhs", bufs=PF + 2))
        mk_pool = ctx.enter_context(tc.tile_pool(name="mk", bufs=PF + 2))
        ps_pool = ctx.enter_context(tc.tile_pool(name="ps", bufs=3, space="PSUM"))
        py_pool = ctx.enter_context(tc.tile_pool(name="py", bufs=1, space="PSUM"))
        r_pool = ctx.enter_context(tc.tile_pool(name="r", bufs=8))
        yh_pool = ctx.enter_context(tc.tile_pool(name="yh", bufs=3))
        ym_pool = ctx.enter_context(tc.tile_pool(name="ym", bufs=3))

        w1 = const.tile([128, 256], BF16)
        nc.sync.dma_start(w1[:, :], w1d[:, :])
        wc = const.tile([128, 64], BF16)
        nc.sync.dma_start(wc[:, :], wcd[:, :])

        # single persistent py psum tile: h0 at cols 0:480, h1 at 512:992
        py = py_pool.tile([128, 1024], F32)

        rhs_tiles = {}
        mk_tiles = {}

        def load_step(t):
            rhs = rhs_pool.tile([128, 2 * CH], BF16, tag="rhs")
            for cg in range(NGRP):
                nc.sync.dma_start(rhs[32 * cg : 32 * cg + 18, :], xym[t, cg])
            mk = mk_pool.tile([128, 1024], BF16, tag="mk")
            mg = mk[:, :].rearrange("(g s) (h c) -> g s h c", s=32, c=512)
            nc.sync.dma_start(mg[:, 18:19, :, 0:CH], msk[t])
            rhs_tiles[t] = rhs
            mk_tiles[t] = mk

        for t in range(min(PF, nt)):
            load_step(t)

        for t in range(nt):
            rhs = rhs_tiles.pop(t)
            mk = mk_tiles.pop(t)
            if t + PF < nt:
                load_step(t + PF)

            K = 18 if t == 0 else 19  # t=0 has no feedback row
            rtiles = {}
            tidx = 0
            for h in range(NHALF):
                # mm1: one K-row matmul per (m, cg); 4-way row-tiled
                for m in range(2):
                    for p in range(2):
                        ps = ps_pool.tile([128, 1024], F32, tag="ps")
                        for q in range(2):
                            cg = 2 * p + q
                            nc.tensor.matmul(
                                ps[:, q * 512 : q * 512 + CH],
                                w1[32 * cg : 32 * cg + K, m * 128 : (m + 1) * 128],
                                rhs[32 * cg : 32 * cg + K, h * CH : (h + 1) * CH],
                                start=True,
                                stop=True,
                                tile_position=(32 * cg, 0),
                                skip_group_check=True,
                            )
                        # relu + PSUM->SBUF evacuation, split ACT/DVE
                        r = r_pool.tile([128, 1024], BF16, tag="r")
                        if tidx % 4 < _ACT_TILES // 2:
                            nc.scalar.activation(r[:, :], ps[:, :], AF.Relu)
                        else:
                            nc.vector.tensor_scalar_max(r[:, :], ps[:, :], 0.0)
                        tidx += 1
                        rtiles[(m, p)] = r

                # mm3: py[band 32cg] = Wc . relu(u); col-tiled 4-way
                for m in range(2):
                    for cg in range(NGRP):
                        p, q = cg >> 1, cg & 1
                        nc.tensor.matmul(
                            py[32 * cg : 32 * cg + 32, h * 512 : h * 512 + CH],
                            wc[:, m * 32 : (m + 1) * 32],
                            rtiles[(m, p)][:, q * 512 : q * 512 + CH],
                            start=(m == 0),
                            stop=(m == 1),
                            tile_position=(0, 32 * cg),
                            skip_group_check=True,
                        )

                # evacuate py -> SBUF bf16 (output AND feedback source)
                if h == 0:
                    yh = yh_pool.tile([128, 1024], BF16, tag="yh")
                nc.scalar.activation(
                    yh[:, h * 512 : (h + 1) * 512], py[:, h * 512 : (h + 1) * 512],
                    AF.Copy,
                )

                if t + 1 < nt:
                    # fb = mask(t+1) * py(t)   (GPSIMD, all-SBUF, off hot engines)
                    if h == 0:
                        ym = ym_pool.tile([128, 1024], BF16, tag="ym")
                    nc.gpsimd.tensor_tensor(
                        ym[:, h * 512 : (h + 1) * 512],
                        yh[:, h * 512 : (h + 1) * 512],
                        mk[:, h * 512 : (h + 1) * 512],
                        op=ALU.mult,
                    )
                    # place fb into rhs(t+1) row 18+32cg (partition-strided DMA)
                    ym_v = ym[:, :].rearrange("(g s) (i c) -> g s i c", s=32, c=512)
                    nxt = rhs_tiles[t + 1][:, :].rearrange(
                        "(g s) (i c) -> g s i c", s=32, c=CH
                    )
                    nc.sync.dma_start(
                        nxt[:, 18:19, h : h + 1, 0:CH],
                        ym_v[:, 18:19, h : h + 1, 0:CH],
                    )

            # output DMA (both halves)
            yv = yh[:, :].rearrange("(g s) (h c) -> g s h c", s=32, c=512)
            nc.sync.dma_start(outd[t], yv[:, 0, :, 0:CH])

    return nc


def _prep_core_inputs(x_c, y_c, bc):
    """x_c (nt, GSH, 16) f32, y_c (nt, GSH) f32 (NaN = missing) ->
    xym (nt,4,18,960), msk (nt,4,2,480) bf16."""
    import ml_dtypes

    nt = x_c.shape[0]
    xym = np.empty((nt, NGRP, 18, 2 * CH), dtype=np.float32)
    # grid index G = (h*NGRP + cg)*CH + j  <->  xym[t, cg, :, h*CH + j]
    xv = x_c.reshape(nt, NHALF, NGRP, CH, NX)  # [t, h, cg, j, f]
    xym[:, :, 1:17, :] = xv.transpose(0, 2, 4, 1, 3).reshape(nt, NGRP, NX, 2 * CH)
    yv = y_c.reshape(nt, NHALF, NGRP, CH)
    y_clean = np.nan_to_num(yv, nan=0.0, posinf=None, neginf=None)
    mask = np.isnan(yv).astype(np.float32)
    y0 = y_clean + bc * mask
    y0[0] = y_clean[0]  # t=0 carry is exactly zero: no bc term
    xym[:, :, 0, :] = y0.transpose(0, 2, 1, 3).reshape(nt, NGRP, 2 * CH)
    xym[:, :, 17, :] = 1.0
    # msk[t] holds mask(t+1) in (cg, h, j) layout
    mk = np.zeros((nt, NGRP, NHALF, CH), dtype=np.float32)
    mk[: nt - 1] = mask[1:].transpose(0, 2, 1, 3)
    return (
        np.ascontiguousarray(xym).astype(ml_dtypes.bfloat16),
        np.ascontiguousarray(mk).astype(ml_dtypes.bfloat16),
    )


def _prep_weights(Wi, bi, Wh, bh, Wo, bo):
    import ml_dtypes

    Wc = (Wo.astype(np.float64) @ Wh.astype(np.float64)).reshape(HIDDEN)
    bc = float(
        (Wo.astype(np.float64) @ bh.astype(np.float64) + bo.astype(np.float64))[0]
    )
    wiy = Wi[:, NX].astype(np.float64)
    # stationary rows per group: {0: wiy, 1..16: Wi16.T, 17: bi, 18: wiy}
    W1full = np.empty((19, HIDDEN), dtype=np.float32)
    W1full[0] = wiy
    W1full[1:17] = Wi[:, :NX].T
    W1full[17] = bi
    W1full[18] = wiy
    w1 = np.zeros((128, 256), dtype=ml_dtypes.bfloat16)
    for cg in range(NGRP):
        w1[32 * cg : 32 * cg + 19] = W1full.astype(ml_dtypes.bfloat16)
    # Wc replicated to 32 stationary columns per block so mm3 writes all
    # 128 psum partitions (strided views then pick any lane in the band)
    wcm = np.zeros((128, 64), dtype=ml_dtypes.bfloat16)
    wcm[:, 0:32] = Wc[:128, None].astype(ml_dtypes.bfloat16)
    wcm[:, 32:64] = Wc[128:, None].astype(ml_dtypes.bfloat16)
    return w1, wcm, bc


def kernel(x, y, Wi, bi, Wh, bh, Wo, bo):
    from concourse.bass_utils import run_bass_kernel_spmd

    x = np.asarray(x, dtype=np.float32)
    y = np.asarray(y, dtype=np.float32)
    nt = x.shape[0]
    ngrid = x.shape[1]

    w1, wcm, bc = _prep_weights(
        np.asarray(Wi, np.float32),
        np.asarray(bi, np.float32),
        np.asarray(Wh, np.float32),
        np.asarray(bh, np.float32),
        np.asarray(Wo, np.float32),
        np.asarray(bo, np.float32),
    )

    gpc = ngrid // NCORES
    in_maps = []
    for c in range(NCORES):
        x_c = np.zeros((nt, GSH, NX), dtype=np.float32)
        y_c = np.zeros((nt, GSH), dtype=np.float32)
        x_c[:, :gpc] = x[:, c * gpc : (c + 1) * gpc, :]
        y_c[:, :gpc] = y[:, c * gpc : (c + 1) * gpc, 0]
        xym, mk = _prep_core_inputs(x_c, y_c, bc)
        in_maps.append({"xym": xym, "msk": mk, "w1": w1, "wc": wcm})

    nc = _build_nc(nt)
    _legalize_sync(nc)
    results = run_bass_kernel_spmd(nc, in_maps, core_ids=list(range(NCORES)))
    global _LAST_EXEC_NS, _LAST_RESULTS
    _LAST_EXEC_NS = results.exec_time_ns
    _LAST_RESULTS = results

    out = np.empty((nt, ngrid, 1), dtype=np.float32)
    for c in range(NCORES):
        # (nt, NGRP, NHALF, CH) -> (nt, GSH):  G = (h*NGRP+cg)*CH + j
        o = results.results[c]["out"].astype(np.float32) + bc
        o = o.transpose(0, 2, 1, 3).reshape(nt, GSH)
        out[:, c * gpc : (c + 1) * gpc, 0] = o[:, :gpc]
    return out
